# revision 1
# baseline (speedup 1.0000x reference)
"""Trainium2 Bass kernel for nn_DDSTSTransformer_9663676416718.

Data-parallel over batch B=32 across 8 cores (4 batches/core), params
replicated. Per-core activations live as [128, 512] tiles = 4 batches
stacked along partitions (32 channels each), which lets every K=32 or
M=32 matmul pack 4-wide into the PE array via tile_position row/col
groups.

Key structural choices:
  - DDS conv (pointwise + softmax-gated depthwise pair) is host-fused
    into a single 15-tap full conv W[t, c, o]; on device it is 15
    accumulating matmuls over shifted free-axis windows of a zero-padded
    input tile (no im2col materialization).
  - q == k in this model, so the score matrix S = K^T K is symmetric.
    We compute S in [keys-on-partitions, queries-on-free] layout, exp it
    on ACT without max subtraction (|S| <= ~1.3), and get the softmax
    denominator Z for free by appending a ones column to V^T in the
    P @ V matmul (row 32 of the output accumulates Z).
  - V is produced directly transposed (vT[l, o]) by swapping which
    operand of the conv matmul is stationary, so no transposes anywhere.
  - Division by Z: Z row -> [128, 16] layout via DMA, one batched DVE
    reciprocal, DMA back to a row, GPSIMD partition_broadcast, one DVE
    tensor-tensor multiply.
  - InstanceNorm rstd via exp(-0.5 * ln(var + eps)) on ACT so only the
    natural_log_exp table set is ever loaded; final sigmoid is computed
    as 1/(1+exp(-x)) for the same reason.
"""

import numpy as np

B, CIN, L = 32, 6, 512
TC, H, D, DM = 32, 8, 6, 4
EPS = 1e-5
NCORES = 8
BPC = B // NCORES  # batches per core
KT = 15            # fused dds conv taps
PAD = (KT - 1) // 2

# wblob per-layer column layout (all fp32, [128, NW]).
# Conv weights are stored for im2col matmuls: partition = (tt, c) with
# tap t = 4*k + tt (padded to 16 taps, tap 15 zero), k in the free dim.
QKOFF = 0                  # [4k, 256] fused qk conv im2col lhsT
VOFF = QKOFF + 4 * 256     # [4k, 256] fused v conv im2col rhs
FF1OFF = VOFF + 4 * 256    # [128] ff1 lhsT, band-replicated
FF2OFF = FF1OFF + 128      # [32]  ff2 lhsT (full 128 partitions)
UOFF = FF2OFF + 32         # [2, 32] unify lhsT per ktile
GBOFF = UOFF + 64          # 4 cols: n1_g, n1_b, n2_g, n2_b (tiled x4)
NW = GBOFF + 4
XPW = L + 16               # padded input width (7 left, 9 right zeros)

_CACHE = {}


def _split_excess_waits(nc, mybir, limits):
    """Walrus's TPB encodings accept a limited number of sync-wait
    commands per instruction (1 for Matmult/LDWEIGHTS on this build).
    Move excess waits onto freshly inserted same-engine NoOps directly
    before the instruction — identical engine-stream semantics, valid
    encoding."""
    for fn in nc.m.functions:
        for blk in fn.blocks:
            il = blk.instructions
            i = 0
            while i < len(il):
                inst = il[i]
                keep = limits.get(type(inst).__name__, 1)
                si = getattr(inst, 'sync_info', None)
                if si is not None and si.on_wait and len(si.on_wait) > keep:
                    waits = list(si.on_wait)
                    extra, rest = waits[:-keep], waits[-keep:]
                    nops = []
                    for w in extra:
                        n = mybir.InstNoOp(name=f'I-wsplit-{nc.next_id()}',
                                           ins=[], outs=[])
                        n.engine = inst.engine
                        n.sync_info = mybir.SyncInfo(on_wait=[w], on_update=[])
                        nops.append(n)
                    inst.sync_info = mybir.SyncInfo(
                        on_wait=rest, on_update=list(si.on_update))
                    il[i:i] = nops
                    i += len(nops)
                i += 1


def _fuse_dds(pw, dwa, dwb, gate):
    """Fold pointwise + gated depthwise pair into W[t, c, o]."""
    g = np.exp(gate - gate.max())
    g = g / g.sum()
    O = pw.shape[0]
    ka, kb = dwa.shape[1], dwb.shape[1]
    k = max(ka, kb)
    wc = np.zeros((O, k), np.float32)
    wc[:, (k - ka) // 2:(k - ka) // 2 + ka] += g[0] * dwa
    wc[:, (k - kb) // 2:(k - kb) // 2 + kb] += g[1] * dwb
    # W[t, c, o] = pw[o, c] * wc[o, t]
    return np.einsum('oc,ot->tco', pw, wc).astype(np.float32)


def _rep4(a):
    """Tile a [32, ...] band 4x along partitions -> [128, ...]."""
    return np.concatenate([a] * 4, axis=0)


def _host_prep(inputs):
    """Build per-core input maps (numpy only)."""
    x = np.asarray(inputs['x'], np.float32)

    wblob = np.zeros((D, 128, NW), np.float32)
    for l in range(D):
        wqk = _fuse_dds(np.asarray(inputs['qk_pw'][l]), np.asarray(inputs['qk_dwa'][l]),
                        np.asarray(inputs['qk_dwb'][l]), np.asarray(inputs['qk_gate'][l]))
        wv = _fuse_dds(np.asarray(inputs['v_pw'][l]), np.asarray(inputs['v_dwa'][l]),
                       np.asarray(inputs['v_dwb'][l]), np.asarray(inputs['v_gate'][l]))
        # [t, c, o] -> partition (tt, c) = 32*tt + c, free (k, o) = 256*k + o
        for W, off in ((wqk, QKOFF), (wv, VOFF)):
            W16 = np.zeros((16, TC, 256), np.float32)
            W16[:KT] = W
            wblob[l, :, off:off + 1024] = (
                W16.reshape(4, 4, TC, 256)        # [k, tt, c, o]
                   .transpose(1, 2, 0, 3)         # [tt, c, k, o]
                   .reshape(128, 1024))
        wblob[l, :, FF1OFF:FF1OFF + 128] = _rep4(np.asarray(inputs['ff_w1'][l]).T)   # [c, m]
        wblob[l, :, FF2OFF:FF2OFF + 32] = np.asarray(inputs['ff_w2'][l]).T           # [dm, o]
        # unify lhsT: u[hc_lo, kt*32+o] = unify_w[o, kt*128+hc_lo]
        ut = np.asarray(inputs['unify_w'][l]).T.reshape(2, 128, TC).transpose(1, 0, 2)
        wblob[l, :, UOFF:UOFF + 64] = ut.reshape(128, 64)
        wblob[l, :, GBOFF + 0] = _rep4(np.asarray(inputs['n1_g'][l]))
        wblob[l, :, GBOFF + 1] = _rep4(np.asarray(inputs['n1_b'][l]))
        wblob[l, :, GBOFF + 2] = _rep4(np.asarray(inputs['n2_g'][l]))
        wblob[l, :, GBOFF + 3] = _rep4(np.asarray(inputs['n2_b'][l]))

    encw = np.zeros((128, TC), np.float32)
    for i in range(4):
        encw[32 * i:32 * i + CIN, :] = np.asarray(inputs['enc_w']).T  # [c, o]

    wlog = _fuse_dds(np.asarray(inputs['log_pw']), np.asarray(inputs['log_dwa']),
                     np.asarray(inputs['log_dwb']), np.asarray(inputs['log_gate']))  # [3, 32, 1]
    wtail = _rep4(wlog[:, :, 0].T)  # [c, t] -> [128, 3]

    in_maps = []
    for core in range(NCORES):
        xin4 = np.zeros((128, L), np.float32)
        for i in range(BPC):
            xin4[32 * i:32 * i + CIN, :] = x[BPC * core + i]
        in_maps.append({
            'xin4': xin4,
            'encw': encw,
            'wblob': wblob,
            'wtail': wtail,
        })
    return in_maps


def build_nc(debug=False, split_waits=True):
    import concourse.bass as bass
    import concourse.mybir as mybir
    import concourse.tile as tile

    FP32 = mybir.dt.float32
    AF = mybir.ActivationFunctionType
    ALU = mybir.AluOpType
    INV_SQRT_C = float(TC) ** -0.5
    F32R = mybir.dt.float32r

    nc = bass.Bass()
    xin4_d = nc.declare_dram_parameter('xin4', [128, L], FP32, isOutput=False)
    encw_d = nc.declare_dram_parameter('encw', [128, TC], FP32, isOutput=False)
    wblob_d = nc.declare_dram_parameter('wblob', [D, 128, NW], FP32, isOutput=False)
    wtail_d = nc.declare_dram_parameter('wtail', [128, 3], FP32, isOutput=False)
    out_d = nc.declare_dram_parameter('out', [BPC, L], FP32, isOutput=True)
    dbg = {}
    if debug:
        for name, shape in [('dbg_enc', [128, L]), ('dbg_qk', [128, L]),
                            ('dbg_vt', [128, 264]), ('dbg_exps', [128, L]),
                            ('dbg_attn', [128, L]), ('dbg_l0', [128, L])]:
            dbg[name] = nc.declare_dram_parameter(name, shape, FP32, isOutput=True)

    with tile.TileContext(nc) as tc:
        with (
            tc.tile_pool(name='pconst', bufs=1) as pconst,
            tc.tile_pool(name='pw', bufs=2) as pw,
            tc.tile_pool(name='pqk', bufs=8) as pqk,
            tc.tile_pool(name='pvt', bufs=16) as pvt,
            tc.tile_pool(name='pexp', bufs=4) as pexp,
            tc.tile_pool(name='pa', bufs=12) as pa,
            tc.tile_pool(name='ppad', bufs=2) as ppad,
            tc.tile_pool(name='pxc', bufs=2) as pxc,
            tc.tile_pool(name='psm', bufs=4) as psm,
            tc.tile_pool(name='pps_s', bufs=4, space='PSUM') as pps_s,
            tc.tile_pool(name='pdram', bufs=6, space='DRAM') as pdram,
        ):

            def mm(out, lhsT, rhs, **kw):
                # float32r streams 1 col/cycle at N>=256 (fp32 is 4x slower)
                nc.tensor.matmul(out=out, lhsT=lhsT.bitcast(F32R),
                                 rhs=rhs.bitcast(F32R), **kw)

            eps_t = pconst.tile([128, 1], FP32, tag='eps')
            nc.vector.memset(eps_t, EPS)
            ones16 = pconst.tile([128, 16], FP32, tag='ones16')
            nc.vector.memset(ones16, 1.0)
            zeros16 = pconst.tile([128, 16], FP32, tag='zeros16')
            nc.vector.memset(zeros16, 0.0)
            xin4 = pconst.tile([128, L], FP32, tag='xin4')
            encw = pconst.tile([128, TC], FP32, tag='encw')
            wtail = pconst.tile([128, 3], FP32, tag='wtail')
            nc.sync.dma_start(out=xin4.bitcast(F32R), in_=xin4_d[:, :].bitcast(F32R))
            nc.sync.dma_start(out=encw.bitcast(F32R), in_=encw_d[:, :].bitcast(F32R))
            nc.sync.dma_start(out=wtail.bitcast(F32R), in_=wtail_d[:, :].bitcast(F32R))

            def load_weights(l):
                w = pw.tile([128, NW], FP32, tag='wsb')
                for a, b in ((QKOFF, VOFF), (VOFF, FF1OFF), (FF1OFF, NW)):
                    nc.sync.dma_start(out=w[:, a:b].bitcast(F32R),
                                      in_=wblob_d[l, :, a:b].bitcast(F32R))
                return w

            wsb = [load_weights(0)]

            # ---------- encoder ----------
            enct = pps_s.tile([128, 2, L], FP32, tag='s', name='enct')
            encp = enct[:, 0, :]
            for i in range(4):
                nc.tensor.matmul(
                    out=encp[32 * i:32 * i + 32, :],
                    lhsT=encw[32 * i:32 * i + 32, :],
                    rhs=xin4[32 * i:32 * i + 32, :],
                    start=True, stop=True,
                    tile_position=(32 * i, 32 * i))
            out4 = pa.tile([128, L], FP32, tag='a')
            nc.vector.tensor_copy(out=out4, in_=encp)
            if debug:
                nc.sync.dma_start(out=dbg['dbg_enc'][:, :], in_=out4)

            def instance_norm(resp_slices, out4_prev, w, goff, boff):
                """res = resp(PSUM slices per band) + out4_prev -> norm."""
                res = pa.tile([128, L], FP32, tag='a')
                for bi, sl in enumerate(resp_slices):
                    nc.vector.tensor_add(out=res[32 * bi:32 * bi + 32, :], in0=sl,
                                         in1=out4_prev[32 * bi:32 * bi + 32, :])
                stats = psm.tile([128, 6], FP32, tag='stats', bufs=2)
                mv = psm.tile([128, 2], FP32, tag='mv', bufs=2)
                nc.vector.bn_stats(out=stats, in_=res)
                nc.vector.bn_aggr(out=mv, in_=stats)
                lnv = psm.tile([128, 1], FP32, tag='lnv', bufs=2)
                rstd = psm.tile([128, 1], FP32, tag='rstd', bufs=2)
                nc.scalar.activation(out=lnv, in_=mv[:, 1:2], func=AF.Ln,
                                     bias=eps_t, scale=1.0)
                nc.scalar.activation(out=rstd, in_=lnv, func=AF.Exp,
                                     bias=0.0, scale=-0.5)
                se = psm.tile([128, 1], FP32, tag='se', bufs=2)
                nc.vector.tensor_mul(out=se, in0=rstd, in1=w[:, goff:goff + 1])
                o = pa.tile([128, L], FP32, tag='a')
                nc.vector.tensor_scalar(out=o.bitcast(F32R), in0=res, scalar1=mv[:, 0:1],
                                        scalar2=se, op0=ALU.subtract, op1=ALU.mult)
                nc.vector.tensor_scalar(out=o.bitcast(F32R), in0=o,
                                        scalar1=w[:, boff:boff + 1],
                                        scalar2=None, op0=ALU.add)
                return o

            def build_xpad(src):
                xp = ppad.tile([128, XPW], FP32, tag='xpad')
                nc.vector.tensor_copy(out=xp[:, 0:PAD].bitcast(F32R),
                                      in_=zeros16[:, 0:PAD])
                nc.vector.tensor_copy(out=xp[:, PAD + L:].bitcast(F32R),
                                      in_=zeros16[:, 0:XPW - PAD - L])
                nc.vector.tensor_copy(out=xp[:, PAD:PAD + L].bitcast(F32R), in_=src)
                return xp

            for l in range(D):
                w = wsb[l]
                if l + 1 < D:
                    wsb.append(load_weights(l + 1))
                xpad = build_xpad(out4)

                # ---------- im2col tiles + DDS convs ----------
                # Tb[b][32*tt + c, k, l] = xpad[c(band b), l + 4k + tt]
                qk_sb = [[None, None] for _ in range(BPC)]
                vt_sb = [[None] * 4 for _ in range(BPC)]
                for bi in range(BPC):
                    xc = pxc.tile([128, 4, L], FP32, tag='xc', name=f'xc{l}_{bi}')
                    for tt in range(4):
                        sl = xpad[32 * bi:32 * bi + 32, tt:tt + L]
                        src_ap = bass.AP(
                            tensor=sl.tensor, offset=sl.offset,
                            ap=[sl.ap[0], [4, 4], [1, L]])
                        nc.sync.dma_start(out=xc[32 * tt:32 * tt + 32, :, :].bitcast(F32R),
                                          in_=src_ap.bitcast(F32R))
                    # qk: [256, 512], otile halves, accumulate over k
                    qp = pps_s.tile([128, 2, L], FP32, tag='s', name=f'qp{l}_{bi}')
                    for j in range(2):
                        for k in range(4):
                            mm(
                                out=qp[:, j, :],
                                lhsT=w[:, QKOFF + k * 256 + j * 128:
                                       QKOFF + k * 256 + j * 128 + 128],
                                rhs=xc[:, k, :],
                                start=(k == 0), stop=(k == 3))
                    for j in range(2):
                        q = pqk.tile([128, L], FP32, tag='qk', name=f'q{l}_{bi}_{j}')
                        nc.vector.tensor_copy(out=q.bitcast(F32R), in_=qp[:, j, :])
                        qk_sb[bi][j] = q
                    # vT: [l, 256] per ltile, accumulate over k
                    vps = [pps_s.tile([128, 2, L], FP32, tag='s', name=f'vp{l}_{bi}_{i}')
                           for i in range(2)]
                    for lt in range(4):
                        vp = vps[lt // 2][:, lt % 2, 0:256]
                        for k in range(4):
                            mm(
                                out=vp,
                                lhsT=xc[:, k, 128 * lt:128 * lt + 128],
                                rhs=w[:, VOFF + k * 256:VOFF + (k + 1) * 256],
                                start=(k == 0), stop=(k == 3))
                    for lt in range(4):
                        v = pvt.tile([128, 264], FP32, tag='vt', name=f'v{l}_{bi}_{lt}')
                        nc.vector.tensor_copy(
                            out=v.rearrange('p (h x) -> p h x', h=H)[:, :, 32:33].bitcast(F32R),
                            in_=ones16[:, 0:H].rearrange('p (a b) -> p a b', b=1))
                        nc.vector.tensor_copy(
                            out=v.rearrange('p (h x) -> p h x', h=H)[:, :, 0:32].bitcast(F32R),
                            in_=vps[lt // 2][:, lt % 2, 0:256].rearrange(
                                'p (h x) -> p h x', h=H))
                        vt_sb[bi][lt] = v
                if debug and l == 0:
                    nc.sync.dma_start(out=dbg['dbg_qk'][:, :], in_=qk_sb[0][0])
                if debug and l == 0:
                    nc.sync.dma_start(out=dbg['dbg_vt'][:, :], in_=vt_sb[0][0])

                # ---------- attention ----------
                attn_sb = [[None, None] for _ in range(BPC)]
                for bi in range(BPC):
                    for g in range(2):  # head group: heads 4g..4g+3
                        qk = qk_sb[bi][g]
                        ek = [pexp.tile([128, 2, 4, L], FP32, tag='exps', name=f'ek{kk}') for kk in range(2)]
                        for k in range(4):
                            psA = pps_s.tile([128, 2, L], FP32, tag='s')
                            psB = pps_s.tile([128, 2, L], FP32, tag='s')
                            for hh in range(4):
                                ps, half = (psA, hh) if hh < 2 else (psB, hh - 2)
                                mm(
                                    out=ps[:, half, :],
                                    lhsT=qk[32 * hh:32 * hh + 32, 128 * k:128 * k + 128],
                                    rhs=qk[32 * hh:32 * hh + 32, :],
                                    start=True, stop=True,
                                    tile_position=(32 * hh, 0))
                            nc.scalar.activation(out=ek[k // 2][:, k % 2, 0:2, :].bitcast(F32R),
                                                 in_=psA[:, :, :], func=AF.Exp,
                                                 bias=0.0, scale=INV_SQRT_C)
                            nc.scalar.activation(out=ek[k // 2][:, k % 2, 2:4, :].bitcast(F32R),
                                                 in_=psB[:, :, :], func=AF.Exp,
                                                 bias=0.0, scale=INV_SQRT_C)
                        if debug and l == 0 and bi == 0 and g == 0:
                            nc.sync.dma_start(out=dbg['dbg_exps'][:, :], in_=ek[0][:, 0, 0, :])

                        # AV: out'[c, m] and Z in row 32, col-tiled 2 heads/bank
                        zrowc = psm.tile([1, 4 * L], FP32, tag='zrowc', bufs=1)
                        zb = psm.tile([128, 16], FP32, tag='zb', bufs=2)
                        avps = []
                        for pp in range(2):  # head pairs (0,1), (2,3)
                            avp = pps_s.tile([128, 2, L], FP32, tag='s',
                                             name=f'avp{l}_{bi}_{g}_{pp}')
                            for k in range(4):
                                for s, hh in enumerate((2 * pp, 2 * pp + 1)):
                                    mm(
                                        out=avp[0:33, s, :],
                                        lhsT=vt_sb[bi][k][:, 33 * (4 * g + hh):33 * (4 * g + hh) + 33],
                                        rhs=ek[k // 2][:, k % 2, hh, :],
                                        start=(k == 0), stop=(k == 3))
                            for s, hh in enumerate((2 * pp, 2 * pp + 1)):
                                nc.vector.tensor_copy(
                                    out=zrowc[0:1, hh * L:(hh + 1) * L],
                                    in_=avp[32:33, s, :])
                            avps.append(avp)
                        nc.sync.dma_start(out=zb, in_=zrowc)
                        rz = psm.tile([128, 16], FP32, tag='rz', bufs=2)
                        nc.vector.reciprocal(out=rz, in_=zb)
                        zdram = pdram.tile([4, L], FP32, tag='zdram')
                        nc.sync.dma_start(out=zdram[:, :], in_=rz)
                        at = pa.tile([128, L], FP32, tag='a')
                        zb32 = psm.tile([32, 4, L], FP32, tag='zb32', bufs=2)
                        bcast_src = bass.AP(
                            tensor=zdram.tensor, offset=zdram.offset,
                            ap=[[0, 32], [L, 4], [1, L]])
                        nc.sync.dma_start(out=zb32, in_=bcast_src)
                        for hh in range(4):
                            avp = avps[hh // 2]
                            s = hh % 2
                            nc.vector.tensor_mul(
                                out=at[32 * hh:32 * hh + 32, :].bitcast(F32R),
                                in0=avp[0:32, s, :],
                                in1=zb32[:, hh, :])
                        attn_sb[bi][g] = at
                if debug and l == 0:
                    nc.sync.dma_start(out=dbg['dbg_attn'][:, :], in_=attn_sb[0][0])

                # ---------- unify (col-tiled over batches) ----------
                upt = [pps_s.tile([128, 2, L], FP32, tag='s', name=f'up{l}_{i}')
                       for i in range(2)]
                for bi in range(BPC):
                    for kt in range(2):
                        mm(
                            out=upt[bi // 2][0:32, bi % 2, :],
                            lhsT=w[:, UOFF + kt * 32:UOFF + (kt + 1) * 32],
                            rhs=attn_sb[bi][kt],
                            start=(kt == 0), stop=(kt == 1))
                up_slices = [upt[bi // 2][0:32, bi % 2, :] for bi in range(BPC)]
                out4 = instance_norm(up_slices, out4, w, GBOFF + 0, GBOFF + 1)

                # ---------- ff ----------
                psA = pps_s.tile([128, 2, L], FP32, tag='s', name=f'ffA{l}')
                psB = pps_s.tile([128, 2, L], FP32, tag='s', name=f'ffB{l}')
                ffsb = []
                for bi in range(BPC):
                    ps, half = (psA, bi) if bi < 2 else (psB, bi - 2)
                    mm(
                        out=ps[:, half, :],
                        lhsT=w[32 * bi:32 * bi + 32, FF1OFF:FF1OFF + 128],
                        rhs=out4[32 * bi:32 * bi + 32, :],
                        start=True, stop=True,
                        tile_position=(32 * bi, 0))
                    f = pa.tile([128, L], FP32, tag='a')
                    nc.vector.tensor_scalar_max(out=f.bitcast(F32R), in0=ps[:, half, :],
                                                scalar1=0.0)
                    ffsb.append(f)
                f2t = [pps_s.tile([128, 2, L], FP32, tag='s', name=f'f2{l}_{i}')
                       for i in range(2)]
                for bi in range(BPC):
                    mm(
                        out=f2t[bi // 2][0:32, bi % 2, :],
                        lhsT=w[:, FF2OFF:FF2OFF + 32],
                        rhs=ffsb[bi],
                        start=True, stop=True)
                f2_slices = [f2t[bi // 2][0:32, bi % 2, :] for bi in range(BPC)]
                out4 = instance_norm(f2_slices, out4, w, GBOFF + 2, GBOFF + 3)
                if debug and l == 0:
                    nc.sync.dma_start(out=dbg['dbg_l0'][:, :], in_=out4)

            # ---------- logits head + sigmoid ----------
            xpad = build_xpad(out4)
            l4c = pa.tile([1, BPC * L], FP32, tag='l4', bufs=1)
            for bi in range(BPC):
                pst = pps_s.tile([128, 2, L], FP32, tag='s', name=f'tail{bi}')
                ps = pst[:, 0, :]
                for t in range(3):
                    mm(
                        out=ps[0:1, :],
                        lhsT=wtail[32 * bi:32 * bi + 32, t:t + 1],
                        rhs=xpad[32 * bi:32 * bi + 32, PAD - 1 + t:PAD - 1 + t + L],
                        start=(t == 0), stop=(t == 2),
                        tile_position=(32 * bi, 0))
                nc.vector.tensor_copy(out=l4c[0:1, bi * L:(bi + 1) * L], in_=ps[0:1, :])
            l4e = pa.tile([1, BPC * L], FP32, tag='l4e', bufs=1)
            nc.scalar.activation(out=l4e, in_=l4c, func=AF.Exp, bias=0.0, scale=-1.0)
            nc.vector.tensor_scalar(out=l4e, in0=l4e, scalar1=1.0, scalar2=None,
                                    op0=ALU.add)
            r128 = psm.tile([128, 16], FP32, tag='r128', bufs=1)
            nc.sync.dma_start(out=r128, in_=l4e)
            rr = psm.tile([128, 16], FP32, tag='rr', bufs=1)
            nc.vector.reciprocal(out=rr, in_=r128)
            nc.sync.dma_start(out=out_d[:, :], in_=rr)

    if split_waits:
        _split_excess_waits(nc, mybir, {'InstNoOp': 99})
    return nc


def _get_nc():
    if 'nc' not in _CACHE:
        _CACHE['nc'] = build_nc(debug=False)
    return _CACHE['nc']


def kernel(**inputs) -> np.ndarray:
    from concourse.bass_utils import run_bass_kernel_spmd

    nc = _get_nc()
    in_maps = _host_prep(inputs)
    res = run_bass_kernel_spmd(nc, in_maps, list(range(NCORES)))
    return np.concatenate([r['out'] for r in res.results], axis=0)

